# revision 16
# baseline (speedup 1.0000x reference)
"""AWQ int4 linear + fused LoRA on 8 Trainium2 NeuronCores.

Computes out = x @ dequant(qweight, qzeros, scales) + (x @ lora_a) @ lora_b
with tensor-parallel column sharding over N (no collectives needed).

Per-core device kernel:
  Phase A: dequantize the per-core weight shard W = q*s + (A@B - z*s) into
    SBUF (bf16).  The LoRA product and the zero-point correction ride ONE
    matmul per tile: lhsT = [A^T ; IND ; IND] (128 x K) against
    rhs = [B ; c_hi ; c_lo] (128 x NP), where IND is the group-indicator
    and c = -z*s is split hi/lo to keep ~16 mantissa bits.  A second
    matmul broadcasts s across partitions; two DVE ops merge:
    W = q * s_bcast + (AB + c).
  Phase B: dense bf16 GEMM x^T-tiles (k-major, pre-cast to bf16 on host,
    HWDGE loads) against the resident W, accumulating fp32 in PSUM
    (full-width 1376-col psum tile, single evict per token tile).
"""

import sys

if "/opt/trn_rl_repo" not in sys.path:
    sys.path.insert(0, "/opt/trn_rl_repo")

import numpy as np

P = 128
N_CORES = 8
T_FULL, K_FULL, N_FULL = 8192, 4096, 11008
R_FULL = 64
NSH = N_FULL // N_CORES  # 1376 columns per core
NP_FULL = NSH


def _n_slices(NP, max_free=512):
    out = []
    off = 0
    while off < NP:
        ns = min(max_free, NP - off)
        out.append((off, ns))
        off += ns
    return out


def _patched_tile_context(tile_mod, nc):
    """TileContext whose tail drain keeps <=1 sem wait per SP instruction
    (this walrus build rejects >2 sync waits on a Drain)."""
    from bass_rust import ScopedClock, SyncInfo

    class TileContextPatched(tile_mod.TileContext):
        def _drain_and_barrier(self, tick_clock, wait_clock):
            drain_inst = self.nc.sync.drain()
            wait_clock.add_sem_waits(
                drain_inst.ins, ScopedClock({None: tick_clock.global_clock})
            )
            si = drain_inst.ins.sync_info
            waits = list(si.on_wait) if si is not None else []
            if len(waits) > 1:
                drain_inst.ins.sync_info.on_wait = waits[:1]
                for w in waits[1:]:
                    nop = self.nc.sync.nop()
                    nop.ins.sync_info = SyncInfo(on_wait=[w], on_update=[])

            self.nc.all_engine_barrier()
            assert self.sems is not None
            popped = self.nc._tile_sem_poison_stack.pop()
            assert popped is self._sem_poison
            self.nc.clear_and_free_semaphores(list(self.sems.allocated().values()))
            self.nc.all_engine_barrier()

    return TileContextPatched(nc)


def _split_multi_waits(nc, max_waits=1):
    """This walrus build rejects instructions carrying more than ~1-2 sem
    waits ('Too many sync wait commands').  Move extra waits onto standalone
    EventSemaphore instructions inserted just before, on the same engine —
    engines execute their stream in order, so this is semantically identical.
    """
    from concourse import mybir

    n_split = 0
    for f in nc.m.functions:
        for bb in f.blocks:
            insts = list(bb.instructions)
            out, changed = [], False
            for inst in insts:
                si = inst.sync_info
                if si is not None and len(si.on_wait) > max_waits:
                    waits = list(si.on_wait)
                    for w in waits[:-max_waits]:
                        n_split += 1
                        nop = mybir.InstEventSemaphore(
                            name=f"{inst.name}-ws{n_split}", ins=[], outs=[])
                        nop.engine = inst.engine
                        nop.sync_info = mybir.SyncInfo(on_wait=[w], on_update=[])
                        out.append(nop)
                    si.on_wait = waits[-max_waits:]
                    changed = True
                out.append(inst)
            if changed:
                bb.instructions = out
    return n_split


def _dedupe_ldweights(nc):
    """Legalization pairs every InstMatmult with its own InstLdweights, even
    when consecutive matmuls share the same stationary operand (our phase B
    issues 3 N-slice matmuls per x-tile).  Weights persist in the PE array
    across matmuls, so a reload of the identical physical AP is pure
    overhead (~P/1.2GHz each on the PE queue).  Drop such repeats, keeping
    their sem waits/updates on a nop so the sync graph is unchanged."""
    from concourse import mybir

    n = 0
    for f in nc.m.functions:
        for bb in f.blocks:
            insts = list(bb.instructions)
            out, changed = [], False
            prev_key = None
            for inst in insts:
                if isinstance(inst, mybir.InstLdweights):
                    key = (str(inst.ins[0]), str(inst.perf_mode),
                           str(inst.is_transpose), str(inst.tile_position),
                           str(inst.tile_size))
                    if key == prev_key:
                        si = inst.sync_info
                        if si is not None and (si.on_wait or si.on_update):
                            nop = mybir.InstEventSemaphore(
                                name=f"{inst.name}-lw", ins=[], outs=[])
                            nop.engine = inst.engine
                            nop.sync_info = si
                            out.append(nop)
                        n += 1
                        changed = True
                        continue
                    prev_key = key
                elif str(getattr(inst, "engine", None)) == "EngineType.PE":
                    if isinstance(inst, mybir.InstMatmult):
                        if getattr(inst, "is_transpose", False):
                            prev_key = None  # transpose uses the weight path
                    elif not isinstance(inst, mybir.InstEventSemaphore):
                        prev_key = None  # drains/branches: be conservative
                out.append(inst)
            if changed:
                bb.instructions = out
    return n


ALL_FEATURES = frozenset({"phase_a", "xload", "mm", "evict", "store"})


def build_bass(T=T_FULL, K=K_FULL, NP=NP_FULL, R=R_FULL, TSUP=256,
               num_devices=N_CORES, split_waits=True, repeat=1,
               loop_repeat=1, features=ALL_FEATURES, xb_bufs=3):
    """Build the per-core Bass program (SPMD: all cores run this)."""
    import concourse.bass as bass
    import concourse.tile as tile
    from concourse import mybir

    NG = K // P  # k-tiles; == quant groups (group size 128)
    assert T % TSUP == 0 and TSUP % P == 0
    f32, bf16 = mybir.dt.float32, mybir.dt.bfloat16

    nc = bass.Bass("TRN2", target_bir_lowering=False, debug=False,
                   num_devices=num_devices)
    xt_d = nc.dram_tensor("xt", [K, T], bf16, kind="ExternalInput")
    q_d = nc.dram_tensor("q", [K, NP], bf16, kind="ExternalInput")
    sx_d = nc.dram_tensor("sx", [2 * (K // P), NP], bf16, kind="ExternalInput")
    atx_d = nc.dram_tensor("atx", [P, K], bf16, kind="ExternalInput")
    bcx_d = nc.dram_tensor("bcx", [P, NP], bf16, kind="ExternalInput")
    out_d = nc.dram_tensor("out", [T, NP], f32, kind="ExternalOutput")

    slices = _n_slices(NP)

    from contextlib import ExitStack

    tc = _patched_tile_context(tile, nc)
    with tc, ExitStack() as ctx:
        const = ctx.enter_context(tc.tile_pool(name="const", bufs=1))
        # lhsT for the LoRA + zero-point correction: [A^T ; IND ; IND]
        atx_sb = const.tile([P, K], bf16, name="atx_sb")
        nc.sync.dma_start(atx_sb[:], atx_d.ap())
        # rhs: [B ; c_hi ; c_lo], host-precomputed
        bcx_sb = const.tile([P, NP], bf16, name="bcx_sb")
        nc.sync.dma_start(bcx_sb[:], bcx_d.ap())
        # scales hi/lo split [s_hi ; s_lo] (2NG x NP) bf16 — the broadcast
        # matmul against [IND ; IND] reconstructs s_hi+s_lo in fp32 PSUM
        # (~16 mantissa bits) at bf16 matmul speed (no f32r self-loading)
        sx_sb = const.tile([2 * NG, NP], bf16, name="sx_sb")
        nc.sync.dma_start(sx_sb[:], sx_d.ap())
        # [IND ; IND] lhsT for the s broadcast: rows R..R+2NG of atx
        indx_sb = const.tile([2 * NG, K], bf16, name="indx_sb")
        nc.sync.dma_start(indx_sb[:], atx_d.ap()[R:R + 2 * NG, :])

        wpool = ctx.enter_context(tc.tile_pool(name="wpool", bufs=1))
        W_sb = wpool.tile([P, NG, NP], bf16, name="W_sb")
        if "phase_a" not in features:
            nc.vector.memset(W_sb[:, 0:1, 0:1], 0.0)

        # All working pools coexist at one scope: phase A and phase B tiles
        # never alias addresses, so the scheduler can overlap the phases.
        deq = ctx.enter_context(tc.tile_pool(name="deq", bufs=2))
        ps_pool = ctx.enter_context(tc.tile_pool(name="ps", bufs=1,
                                                 space="PSUM"))
        xb = ctx.enter_context(tc.tile_pool(name="xb", bufs=xb_bufs))
        ob = ctx.enter_context(tc.tile_pool(name="ob", bufs=2))

        from contextlib import nullcontext

        for rep in range(repeat):
          # loop_repeat>1 re-runs the body via a hardware loop (constant
          # program size; used by the timing harness for slope measurement)
          with (tc.For_i(0, loop_repeat, 1) if loop_repeat > 1
                else nullcontext()):
            # ---- Phase A: dequant + LoRA fold ----
            if "phase_a" in features:
                for j in range(NG):
                    q_t = deq.tile([P, NP], bf16, name="q_t")
                    # NOTE: routing this load off the SP HWDGE ring was tried
                    # both ways and rejected: via nc.scalar it costs ACT
                    # sequencer time that delays the sb_t evicts (+28us in
                    # the cost model); via nc.gpsimd (SWDGE) walrus fails to
                    # compile the program.  SP ring it is.
                    nc.sync.dma_start(q_t[:], q_d.ap()[j * P:(j + 1) * P, :])
                    wj = W_sb[:, j]
                    for (off, ns) in slices:
                        # broadcast s_j across partitions: psum[p,n] = s[j,n]
                        ps_s = ps_pool.tile([P, 512], f32, name="ps_s",
                                            bufs=1)
                        nc.tensor.matmul(
                            ps_s[:, :ns],
                            lhsT=indx_sb[:, j * P:(j + 1) * P],
                            rhs=sx_sb[:, off:off + ns],
                            start=True, stop=True,
                        )
                        # LoRA chunk + zero-point: [A^T;IND;IND]^T @ [B;c]
                        ps_ab = ps_pool.tile([P, 512], f32, name="ps_ab",
                                             bufs=1)
                        nc.tensor.matmul(
                            ps_ab[:, :ns],
                            lhsT=atx_sb[:, j * P:(j + 1) * P],
                            rhs=bcx_sb[:, off:off + ns],
                            start=True, stop=True,
                        )
                        # evict broadcast to SBUF bf16 on the (idle) ACT
                        # engine so the DVE multiply runs in 2x bf16 mode
                        sb_t = deq.tile([P, 512], bf16, name="sb_t")
                        nc.scalar.copy(sb_t[:, :ns], ps_s[:, :ns])
                        qs_t = deq.tile([P, 512], bf16, name="qs_t")
                        nc.vector.tensor_mul(qs_t[:, :ns],
                                             q_t[:, off:off + ns],
                                             sb_t[:, :ns])
                        # W = qs + (A@B + c) chunk (single fp32-sourced add)
                        nc.vector.tensor_add(wj[:, off:off + ns],
                                             qs_t[:, :ns],
                                             ps_ab[:, :ns])

            # ---- Phase B: main GEMM ----
            if True:
                xt_r = xt_d.ap().rearrange("(j p) t -> p j t", p=P)
                for sidx in range(T // TSUP):
                    t0 = sidx * TSUP
                    x_t = xb.tile([P, NG, TSUP], bf16, name="x_t")
                    if "xload" in features:
                        nc.sync.dma_start(x_t[:], xt_r[:, :, t0:t0 + TSUP])
                    else:
                        nc.vector.memset(x_t[:, 0:1, 0:1], 0.0)
                    for tsub in range(TSUP // P):
                        pt = ps_pool.tile([P, NP], f32, name="mm_ps",
                                          bufs=2)
                        if "mm" in features:
                            for j in range(NG):
                                lhsT = x_t[:, j][:, tsub * P:(tsub + 1) * P]
                                for (off, ns) in slices:
                                    nc.tensor.matmul(
                                        pt[:, off:off + ns],
                                        lhsT=lhsT,
                                        rhs=W_sb[:, j][:, off:off + ns],
                                        start=(j == 0),
                                        stop=(j == NG - 1),
                                    )
                        out_t = ob.tile([P, NP], f32, name="out_t")
                        if "evict" in features and "mm" in features:
                            nc.vector.tensor_copy(out_t[:], pt[:])
                        else:
                            nc.vector.memset(out_t[:, 0:1], 0.0)
                        if "store" in features:
                            nc.sync.dma_start(
                                out_d.ap()[t0 + tsub * P:t0 + (tsub + 1) * P, :],
                                out_t[:],
                            )
    # NOTE: an LDW-dedupe pass (drop repeated identical InstLdweights, keeping
    # one per 3-matmul slice group) was tried here and VERIFIED WRONG on HW:
    # rel err jumped to 7.4e-2 with no speedup — the per-matmul LDW pairing
    # is required by this toolchain/silicon and is already overlap-hidden.
    if split_waits:
        _split_multi_waits(nc)
    return nc


def _marshal_inputs(x, scales, lora_a, lora_b, qweight, qzeros,
                    n_cores=N_CORES, NP=NP_FULL):
    """Host-side sharding + layout prep (transpose / dtype cast / padding)."""
    import ml_dtypes

    bf16 = ml_dtypes.bfloat16
    x = np.asarray(x, dtype=np.float32)
    scales = np.asarray(scales, dtype=np.float32)
    lora_a = np.asarray(lora_a, dtype=np.float32)
    lora_b = np.asarray(lora_b, dtype=np.float32)
    qweight = np.asarray(qweight, dtype=np.int32)
    qzeros = np.asarray(qzeros, dtype=np.int32)

    K, N = qweight.shape
    NG = scales.shape[0]
    nsh = N // n_cores

    xt = np.ascontiguousarray(x.T).astype(bf16)     # [K, T] bf16
    indic = np.kron(np.eye(NG, dtype=np.float32),
                    np.ones((1, P), np.float32))    # [NG, NG*128]
    # lhsT [A^T ; IND ; IND]  (128 x K) bf16 — IND rows are exact in bf16
    atx = np.concatenate([lora_a.T, indic, indic], axis=0).astype(bf16)

    # zero-point correction c = -z*s, split hi+lo for ~16 mantissa bits
    c = -(qzeros.astype(np.float32) * scales)       # [NG, N]
    c_hi = c.astype(bf16)
    c_lo = (c - c_hi.astype(np.float32)).astype(bf16)

    # scales split hi+lo the same way (reconstructed as s_hi+s_lo in fp32
    # PSUM by the broadcast matmul, keeping ~16 mantissa bits)
    s_hi = scales.astype(bf16)
    s_lo = (scales - s_hi.astype(np.float32)).astype(bf16)

    qb = qweight.astype(bf16)                       # exact (values 0..15)

    in_maps = []
    for cidx in range(n_cores):
        lo, hi = cidx * nsh, (cidx + 1) * nsh
        q = np.ascontiguousarray(qb[:, lo:hi])
        sx = np.ascontiguousarray(
            np.concatenate([s_hi[:, lo:hi], s_lo[:, lo:hi]], axis=0))
        bcx = np.concatenate([lora_b[:, lo:hi].astype(bf16),
                              c_hi[:, lo:hi], c_lo[:, lo:hi]], axis=0)
        bcx = np.ascontiguousarray(bcx)             # [128, nsh] bf16
        in_maps.append({"xt": xt, "q": q, "sx": sx, "atx": atx, "bcx": bcx})
    return in_maps, nsh


_NC_CACHE = {}


def kernel(x, scales, lora_a, lora_b, qweight, qzeros):
    from concourse.bass_utils import run_bass_kernel_spmd

    in_maps, nsh = _marshal_inputs(x, scales, lora_a, lora_b, qweight, qzeros)
    key = "full"
    if key not in _NC_CACHE:
        _NC_CACHE[key] = build_bass()
    nc = _NC_CACHE[key]
    res = run_bass_kernel_spmd(nc, in_maps, core_ids=list(range(N_CORES)),
                               trace=False)
    outs = [res.results[c]["out"] for c in range(N_CORES)]
    return np.ascontiguousarray(np.concatenate(outs, axis=1))
